# revision 44
# baseline (speedup 1.0000x reference)
"""Trainium2 Bass kernel for nn_ByteEncoder (multi-scale conv stem + per-channel LRU).

Sharding: 8 cores = (batch b in 0..3) x (time-half h in 0..1). Each core runs an
identical SPMD program over raw steps [t0-128, t0+4096) (t0 = h*4096), i.e. a
32-scan-step warmup plus its 1024 output scan steps. The warmup region is
masked to zero for h=0 cores (reference scan starts at state 0) and uses real
left-context for h=1 cores (per-channel decay lambda^32 < 1.5e-6, far below the
2e-2 tolerance).

The embedding lookup is algebraically fused into the conv stem: for one-hot
inputs, conv_k(embed[x]) == sum_taps (embed @ conv_w[:,:,j])[x[t+off]], so the
stem becomes matmuls of precontracted [256-vocab x 256-ch] tables against
one-hot columns built on-chip (iota + is_equal).

All matmuls run in bf16 (full PE rate, fast-weight-load path, half the SBUF and
HBM traffic of fp32). Everything stays in SBUF: stem and the strided down-conv
are fused per 512-step tile, and phase 3/4 (LN -> b-proj -> scan -> c-proj ->
LN) interleaves with later stem tiles so the tensor engine never idles. The
short warmup tile computes its down-conv transposed (cheap N=32 matmuls) and
normalizes across partitions via ones-matmul stats + partition_broadcast.
"""
import numpy as np

import concourse.bass as bass
import concourse.tile as tile
from concourse import mybir, bacc
from concourse.bass_utils import run_bass_kernel_spmd
from concourse.masks import make_identity

P = 128
D = 1024
B = 4
T = 8192
VOCAB = 256
SENTINEL = 512.0  # out-of-range token -> one-hot col is all zero

W_SCAN = 32             # warmup scan steps (lam^32 < 1.5e-6)
S_LOC = 1024 + W_SCAN   # scan steps computed per core
T_LOC = 4 * S_LOC       # raw steps per core (4224)
X_LOC = T_LOC + 8       # x slice incl conv halo (left 4, right 3, +1 pad)
N_CH = 8                # output chunks c1..c8 (128 scan steps each)
W_MAX = 384

f32 = mybir.dt.float32
bf16 = mybir.dt.bfloat16
AF = mybir.ActivationFunctionType
OP = mybir.AluOpType

# scan groups, in scan columns: g0 = warmup(32) + chunks 1-2, then 3/2/1 chunks.
# Last group is a single chunk so the serial tail after the last stem tile is
# as short as possible.
GROUPS = [(32 + 256), 384, 256, 128]
# chunk k (1..8) -> (group, col offset inside group)
CHUNK_POS = {1: (0, 32), 2: (0, 160), 3: (1, 0), 4: (1, 128), 5: (1, 256),
             6: (2, 0), 7: (2, 128), 8: (3, 0)}

# (conv_id, kernel_size, pad); tap offset = j - pad
CONVS = [(1, 0), (2, 1), (4, 2), (8, 4)]
TAPS = []  # (conv_id, j, off)
for ci, (K, pad) in enumerate(CONVS):
    for j in range(K):
        TAPS.append((ci, j, j - pad))
N_TAPS = len(TAPS)  # 15
TAPS_OF_CONV = [[kk for kk, (ci, _, _) in enumerate(TAPS) if ci == c] for c in range(4)]

_CACHE = {}


def _build():
    nc = bacc.Bacc()

    x_d = nc.declare_dram_parameter("x_loc", [X_LOC], bf16, isOutput=False)
    maskr_d = nc.declare_dram_parameter("mask_row", [W_MAX], bf16, isOutput=False)
    stem_d = nc.declare_dram_parameter("stem_w", [2, P, N_TAPS, 256], bf16, isOutput=False)
    convb_d = nc.declare_dram_parameter("convb", [P, 8], f32, isOutput=False)
    dw_d = nc.declare_dram_parameter("down_wt", [P, 4, 8, D], bf16, isOutput=False)
    downbr_d = nc.declare_dram_parameter("downb_v", [D], bf16, isOutput=False)
    downbT_d = nc.declare_dram_parameter("downbT", [P, 8], f32, isOutput=False)
    bw_d = nc.declare_dram_parameter("b_wt", [P, 8, D], bf16, isOutput=False)
    bb2_d = nc.declare_dram_parameter("bb2", [P, 8], f32, isOutput=False)
    bb2r_d = nc.declare_dram_parameter("bb2_row", [D], bf16, isOutput=False)
    cw_d = nc.declare_dram_parameter("c_wt", [P, 8, D], bf16, isOutput=False)
    slnw_d = nc.declare_dram_parameter("slnw_v", [D], bf16, isOutput=False)
    ccb_d = nc.declare_dram_parameter("ccb_v", [D], bf16, isOutput=False)
    lruw_d = nc.declare_dram_parameter("lruw_v", [D], bf16, isOutput=False)
    lrub_d = nc.declare_dram_parameter("lrub_v", [D], bf16, isOutput=False)
    lam_d = nc.declare_dram_parameter("lam_ct", [P, 8], f32, isOutput=False)

    out_d = nc.declare_dram_parameter("out", [1024, D], bf16, isOutput=True)

    with tile.TileContext(nc) as tc:
        with tc.tile_pool(name="glob", bufs=1) as glob, \
             tc.tile_pool(name="pw", bufs=1) as pw, \
             tc.tile_pool(name="p12t", bufs=2) as p12t, \
             tc.tile_pool(name="p34t", bufs=2) as p34t, \
             tc.tile_pool(name="ps_stem", bufs=2, space="PSUM") as ps_stem, \
             tc.tile_pool(name="ps_down", bufs=2, space="PSUM") as ps_down, \
             tc.tile_pool(name="ps_tr", bufs=1, space="PSUM") as ps_tr, \
             tc.tile_pool(name="ps_bp", bufs=1, space="PSUM") as ps_bp, \
             tc.tile_pool(name="ps_cp", bufs=2, space="PSUM") as ps_cp:

            # ---------------- PE warm spin ------------------------------
            # ~12us of junk matmuls at kernel start: trips the PE HAM
            # activity window and keeps it warm while the first weight DMAs
            # land, so real matmuls start at the full 2.4 GHz clock. The
            # operand is a DVE-memset scratch (ready ~5us before the gpsimd
            # identity would be).
            warm_sb = p12t.tile([P, 512], bf16, name="warm_sb", bufs=1)
            nc.vector.memset(warm_sb[:], 0.25)
            warm_ps = ps_cp.tile([P, 512], f32, name="psc", tag="psc")
            for _ in range(33):
                nc.tensor.matmul(warm_ps[:], warm_sb[:, :128], warm_sb[:],
                                 start=True, stop=True)

            # ---------------- critical-path DMAs first -------------------
            stem_sb0 = pw.tile([P, N_TAPS, 256], bf16, name="stem_sb0")
            stem_sb1 = pw.tile([P, N_TAPS, 256], bf16, name="stem_sb1")
            stem_sbs = (stem_sb0, stem_sb1)

            # table slices land in exactly the order the stem's cc groups
            # consume them: cc0 needs just tap 0 / ch-half 0 of both vocab
            # tables (64KB), not the full 2MB
            def stem_dma(cc):
                ci, half = cc // 2, cc % 2
                klo, khi = TAPS_OF_CONV[ci][0], TAPS_OF_CONV[ci][-1] + 1
                for vc in range(2):
                    nc.sync.dma_start(
                        stem_sbs[vc][:, klo:khi, half * 128:(half + 1) * 128],
                        stem_d[vc][:, klo:khi, half * 128:(half + 1) * 128])

            stem_dma(0)

            eps_sb = glob.tile([P, 1], f32, name="eps_sb")
            nc.vector.memset(eps_sb[:], 1e-5)
            ones128 = glob.tile([P, 1], bf16, name="ones128")
            nc.vector.memset(ones128[:], 1.0)
            ones_row = glob.tile([1, 128], bf16, name="ones_row")
            nc.vector.memset(ones_row[:], 1.0)
            ident = glob.tile([P, P], bf16, name="ident")
            make_identity(nc, ident)
            io0 = glob.tile([P, 1], f32, name="io0")
            io1 = glob.tile([P, 1], f32, name="io1")
            nc.gpsimd.iota(io0[:], pattern=[[0, 1]], base=0, channel_multiplier=1,
                           allow_small_or_imprecise_dtypes=True)
            nc.gpsimd.iota(io1[:], pattern=[[0, 1]], base=128, channel_multiplier=1,
                           allow_small_or_imprecise_dtypes=True)

            x_reps = {}

            def issue_xrep(tt):
                x_rep = p12t.tile([P, 520], bf16, name="x_rep", bufs=2)
                if tt == 0:
                    nc.sync.dma_start(
                        x_rep[:, :136],
                        x_d[0:136][None, :].to_broadcast([P, 136]))
                else:
                    lo = 512 * tt - 384
                    nc.sync.dma_start(
                        x_rep[:],
                        x_d[lo: lo + 520][None, :].to_broadcast([P, 520]))
                x_reps[tt] = x_rep

            issue_xrep(0)
            stem_dma(1)
            stem_dma(2)
            convb_sb = glob.tile([P, 8], f32, name="convb_sb")
            nc.sync.dma_start(convb_sb[:], convb_d[:])
            stem_dma(3)
            issue_xrep(1)
            stem_dma(4)
            stem_dma(5)
            stem_dma(6)
            stem_dma(7)

            # ---------------- remaining SBUF state -----------------------
            z_bf = glob.tile([P, N_CH, D], bf16, name="z_bf")
            lam_sb = glob.tile([P, 8], f32, name="lam_sb")
            bb2_sb = glob.tile([P, 8], f32, name="bb2_sb")
            mask_row = glob.tile([1, W_MAX], bf16, name="mask_row")
            bb2_row = glob.tile([1, D], bf16, name="bb2_row")
            downb_rep = glob.tile([P, D], bf16, name="downb_rep")
            dw_sb = pw.tile([P, 4, 8, D], bf16, name="dw_sb")
            bw_sb = pw.tile([P, 8, D], bf16, name="bw_sb")
            cw_sb = pw.tile([P, 8, D], bf16, name="cw_sb")
            slnw_rep = pw.tile([P, D], bf16, name="slnw_rep")
            ccb_rep = pw.tile([P, D], bf16, name="ccb_rep")
            lruw_rep = pw.tile([P, D], bf16, name="lruw_rep")
            lrub_rep = pw.tile([P, D], bf16, name="lrub_rep")

            hm_ts = {}
            hm0 = p12t.tile([P, 8, 128], bf16, name="hm0", bufs=1)

            def stem(tt):
                wid = 128 if tt == 0 else 512
                x_rep = x_reps.pop(tt)
                oh = p12t.tile([P, 2, 520], bf16, name="oh", bufs=2)
                nc.vector.tensor_scalar(out=oh[:, 0, :wid + 8],
                                        in0=x_rep[:, :wid + 8],
                                        scalar1=io0[:], scalar2=None,
                                        op0=OP.is_equal)
                nc.vector.tensor_scalar(out=oh[:, 1, :wid + 8],
                                        in0=x_rep[:, :wid + 8],
                                        scalar1=io1[:], scalar2=None,
                                        op0=OP.is_equal)
                if tt == 0:
                    hm_t = hm0
                else:
                    hm_t = p12t.tile([P, 8, 512], bf16, name="hm_t", bufs=2)
                    hm_ts[tt] = hm_t
                for cc in range(8):
                    ci, half = cc // 2, cc % 2
                    taps = TAPS_OF_CONV[ci]
                    ps = ps_stem.tile([P, 512], f32, name="pss", tag="pss")
                    n_mm = len(taps) * 2
                    i = 0
                    for vc in range(2):
                        for kk in taps:
                            off = TAPS[kk][2]
                            nc.tensor.matmul(
                                ps[:, :wid],
                                stem_sbs[vc][:, kk, half * 128:(half + 1) * 128],
                                oh[:, vc, 4 + off: 4 + off + wid],
                                start=(i == 0), stop=(i == n_mm - 1))
                            i += 1
                    nc.scalar.activation(hm_t[:, cc, :wid], ps[:, :wid],
                                         AF.Gelu, bias=convb_sb[:, cc:cc + 1])

            hd_ts = {}

            def down(c):
                """Standard down-conv for chunk c (tile c), c = 1..8."""
                hm_sb = hm_ts.pop(c)
                hd_t = p34t.tile([P, D], bf16, name="hd_t", tag="hd", bufs=2)
                hd_ts[c] = hd_t
                for eh in range(2):
                    ps = ps_down.tile([P, 512], f32, name="psd", tag="psd")
                    i = 0
                    for dc in range(8):
                        for j in range(4):
                            nc.tensor.matmul(
                                ps[:],
                                hm_sb[:, dc, j:512:4],
                                dw_sb[:, j, dc, eh * 512:(eh + 1) * 512],
                                start=(i == 0), stop=(i == 31))
                            i += 1
                    nc.vector.tensor_tensor(
                        out=hd_t[:, eh * 512:(eh + 1) * 512], in0=ps[:],
                        in1=downb_rep[:, eh * 512:(eh + 1) * 512], op=OP.add)

            def down0_ln0():
                """Warmup (32 scan steps): transposed down-conv (out [e, s],
                cheap N=32 matmuls), partition-axis LN via ones-matmul stats,
                result written straight into hsT group 0 cols 0:32."""
                hdT0 = p34t.tile([P, 8, 32], bf16, name="hdT0", bufs=1)
                for eb in range(8):
                    ps = ps_down.tile([P, 512], f32, name="psd", tag="psd")
                    i = 0
                    for dc in range(8):
                        for j in range(4):
                            nc.tensor.matmul(
                                ps[:128, :32],
                                dw_sb[:, j, dc, eb * 128:(eb + 1) * 128],
                                hm0[:, dc, j:128:4],
                                start=(i == 0), stop=(i == 31))
                            i += 1
                    # add down bias for these 128 e-channels, transposed:
                    # downb slice as per-partition scalar
                    nc.vector.tensor_scalar(
                        out=hdT0[:, eb, :], in0=ps[:128, :32],
                        scalar1=downbT[:, eb:eb + 1], scalar2=None, op0=OP.add)
                sq0 = p34t.tile([P, 8, 32], bf16, name="sq0", bufs=1)
                nc.vector.tensor_tensor(out=sq0[:], in0=hdT0[:], in1=hdT0[:],
                                        op=OP.mult)
                ps = ps_bp.tile([P, W_MAX], f32, name="psb", tag="psb")
                for eb in range(8):
                    nc.tensor.matmul(ps[0:1, 0:32], ones128[:], hdT0[:, eb, :],
                                     start=(eb == 0), stop=(eb == 7))
                for eb in range(8):
                    nc.tensor.matmul(ps[0:1, 64:96], ones128[:], sq0[:, eb, :],
                                     start=(eb == 0), stop=(eb == 7))
                m_row = p34t.tile([1, 32], f32, name="m_row", bufs=1)
                nc.vector.tensor_scalar(out=m_row[:], in0=ps[0:1, 0:32],
                                        scalar1=1.0 / D, scalar2=None,
                                        op0=OP.mult)
                v_row = p34t.tile([1, 32], f32, name="v_row", bufs=1)
                nc.vector.tensor_scalar(out=v_row[:], in0=ps[0:1, 64:96],
                                        scalar1=1.0 / D, scalar2=None,
                                        op0=OP.mult)
                msq = p34t.tile([1, 32], f32, name="msq", bufs=1)
                nc.vector.tensor_tensor(out=msq[:], in0=m_row[:], in1=m_row[:],
                                        op=OP.mult)
                nc.vector.tensor_tensor(out=v_row[:], in0=v_row[:], in1=msq[:],
                                        op=OP.subtract)
                nc.scalar.activation(v_row[:], v_row[:], AF.Sqrt,
                                     bias=eps_sb[0:1, :])
                nc.vector.reciprocal(v_row[:], v_row[:])
                # zero the warmup on h=0 cores
                nc.vector.tensor_tensor(out=v_row[:], in0=v_row[:],
                                        in1=mask_row[:, 0:32], op=OP.mult)
                m_rep = p34t.tile([P, 32], f32, name="m_rep", bufs=1)
                r_rep = p34t.tile([P, 32], f32, name="r_rep", bufs=1)
                nc.gpsimd.partition_broadcast(m_rep[:], m_row[:])
                nc.gpsimd.partition_broadcast(r_rep[:], v_row[:])
                hsT_g = hsT_tiles[0]
                for eb in range(8):
                    zt = p34t.tile([P, 32], f32, name="zt", bufs=2)
                    nc.vector.tensor_tensor(out=zt[:], in0=hdT0[:, eb, :],
                                            in1=m_rep[:], op=OP.subtract)
                    nc.vector.tensor_tensor(out=hsT_g[:, eb, 0:32], in0=zt[:],
                                            in1=r_rep[:], op=OP.mult)

            def lnt(k):
                """LN stats + z + transpose for chunk k (k = 1..8)."""
                g, off = CHUNK_POS[k]
                hd_t = hd_ts.pop(k)
                stats = p34t.tile([P, 2, 6], f32, name="stats", bufs=2)
                hd_g = hd_t[:].rearrange("p (g f) -> p g f", g=2)
                nc.vector.bn_stats(out=stats[:, 0, :], in_=hd_g[:, 0, :])
                nc.vector.bn_stats(out=stats[:, 1, :], in_=hd_g[:, 1, :])
                mv = p34t.tile([P, 2], f32, name="mv", bufs=2)
                nc.vector.bn_aggr(out=mv[:], in_=stats[:])
                rstd = p34t.tile([P, 1], f32, name="rstd", bufs=2)
                nc.scalar.activation(rstd[:], mv[:, 1:2], AF.Sqrt, bias=eps_sb[:])
                nc.vector.reciprocal(rstd[:], rstd[:])
                nc.vector.tensor_scalar(out=z_bf[:, k - 1, :], in0=hd_t[:],
                                        scalar1=mv[:, 0:1], scalar2=rstd[:],
                                        op0=OP.subtract, op1=OP.mult)
                pst = ps_tr.tile([P, 8, P], bf16, name="pst", tag="pst")
                for ec in range(8):
                    nc.tensor.transpose(
                        pst[:, ec, :], z_bf[:, k - 1, ec * 128:(ec + 1) * 128],
                        ident[:])
                hsT_g = hsT_tiles[g]
                nc.scalar.copy(hsT_g[:, :, off:off + 128], pst[:])

            hsT_tiles = {}
            h_tiles = {}

            def open_group(g):
                hsT_tiles[g] = p34t.tile([P, 8, W_MAX], bf16, name="hsT",
                                         tag="hsT", bufs=2)

            def bproj_scan(g):
                W = GROUPS[g]
                hsT_g = hsT_tiles[g]
                h_g = p34t.tile([P, 8, W_MAX], bf16, name="h_g", tag="h_g",
                                bufs=2)
                h_tiles[g] = h_g
                last = (g == len(GROUPS) - 1)
                for dc in range(8):
                    psb = ps_bp.tile([P, W_MAX], f32, name="psb", tag="psb")
                    if g == 0:
                        # masked per-channel bias via 1-row matmul
                        nc.tensor.matmul(psb[:, :W],
                                         bb2_row[:, dc * 128:(dc + 1) * 128],
                                         mask_row[:, :W],
                                         start=True, stop=False)
                    elif last:
                        # bias in psum so the scan can read psum directly
                        # (keeps the vals copy off the tail critical path)
                        nc.tensor.matmul(psb[:, :W],
                                         bb2_row[:, dc * 128:(dc + 1) * 128],
                                         ones_row[:, :W],
                                         start=True, stop=False)
                    for ec in range(8):
                        nc.tensor.matmul(
                            psb[:, :W],
                            bw_sb[:, ec, dc * 128:(dc + 1) * 128],
                            hsT_g[:, ec, :W],
                            start=(g != 0 and not last and ec == 0),
                            stop=(ec == 7))
                    init = (0.0 if g == 0
                            else h_tiles[g - 1][:, dc,
                                                GROUPS[g - 1] - 1: GROUPS[g - 1]])
                    if last:
                        data1 = psb[:, :W]
                    else:
                        vals = p34t.tile([P, W_MAX], bf16, name="vals", bufs=2)
                        if g == 0:
                            nc.vector.tensor_scalar(out=vals[:, :W],
                                                    in0=psb[:, :W], scalar1=0.0,
                                                    scalar2=None, op0=OP.add)
                        else:
                            nc.vector.tensor_scalar(out=vals[:, :W],
                                                    in0=psb[:, :W],
                                                    scalar1=bb2_sb[:, dc:dc + 1],
                                                    scalar2=None, op0=OP.add)
                        data1 = vals[:, :W]
                    nc.vector.tensor_tensor_scan(
                        out=h_g[:, dc, :W],
                        data0=lam_sb[:, dc:dc + 1].to_broadcast([P, W]),
                        data1=data1,
                        initial=init, op0=OP.mult, op1=OP.add)

            def p4(k):
                """c-proj + residual + final LN -> out rows (k-1)*128.."""
                g, off = CHUNK_POS[k]
                h_g = h_tiles[g]
                res_b = p34t.tile([P, D], bf16, name="res_b", bufs=2)
                nc.gpsimd.tensor_tensor(out=res_b[:], in0=z_bf[:, k - 1, :],
                                        in1=slnw_rep[:], op=OP.mult)
                nc.gpsimd.tensor_tensor(out=res_b[:], in0=res_b[:],
                                        in1=ccb_rep[:], op=OP.add)
                res_f = p34t.tile([P, D], bf16, name="res_f", bufs=2)
                for eh in range(2):
                    psc = ps_cp.tile([P, 512], f32, name="psc", tag="psc")
                    for dc in range(8):
                        nc.tensor.matmul(
                            psc[:],
                            h_g[:, dc, off:off + 128],
                            cw_sb[:, dc, eh * 512:(eh + 1) * 512],
                            start=(dc == 0), stop=(dc == 7))
                    nc.vector.tensor_tensor(
                        out=res_f[:, eh * 512:(eh + 1) * 512], in0=psc[:],
                        in1=res_b[:, eh * 512:(eh + 1) * 512], op=OP.add)
                stats2 = p34t.tile([P, 2, 6], f32, name="stats2", bufs=2)
                res_g = res_f[:].rearrange("p (g f) -> p g f", g=2)
                nc.vector.bn_stats(out=stats2[:, 0, :], in_=res_g[:, 0, :])
                nc.vector.bn_stats(out=stats2[:, 1, :], in_=res_g[:, 1, :])
                mv2 = p34t.tile([P, 2], f32, name="mv2", bufs=2)
                nc.vector.bn_aggr(out=mv2[:], in_=stats2[:])
                rstd2 = p34t.tile([P, 1], f32, name="rstd2", bufs=2)
                nc.scalar.activation(rstd2[:], mv2[:, 1:2], AF.Sqrt,
                                     bias=eps_sb[:])
                nc.vector.reciprocal(rstd2[:], rstd2[:])
                nc.vector.tensor_scalar(out=res_f[:], in0=res_f[:],
                                        scalar1=mv2[:, 0:1], scalar2=rstd2[:],
                                        op0=OP.subtract, op1=OP.mult)
                nc.vector.tensor_tensor(out=res_f[:], in0=res_f[:],
                                        in1=lruw_rep[:], op=OP.mult)
                # final bias add + store in halves: the first half's DMA
                # overlaps the second half's DVE op
                for eh in range(2):
                    sl = slice(eh * 512, (eh + 1) * 512)
                    nc.vector.tensor_tensor(out=res_f[:, sl], in0=res_f[:, sl],
                                            in1=lrub_rep[:, sl], op=OP.add)
                    nc.sync.dma_start(out_d[(k - 1) * 128: k * 128, sl],
                                      res_f[:, sl])

            # ---------------- software-pipelined emission ----------------
            open_group(0)
            stem(0)
            issue_xrep(2)
            nc.sync.dma_start(dw_sb[:, 0, :, :], dw_d[:, 0, :, :])
            nc.sync.dma_start(dw_sb[:, 1, :, :], dw_d[:, 1, :, :])
            stem(1)
            issue_xrep(3)
            nc.sync.dma_start(dw_sb[:, 2, :, :], dw_d[:, 2, :, :])
            nc.sync.dma_start(dw_sb[:, 3, :, :], dw_d[:, 3, :, :])
            # small params (needed from phase 3 on)
            nc.sync.dma_start(lam_sb[:], lam_d[:])
            nc.sync.dma_start(bb2_sb[:], bb2_d[:])
            nc.sync.dma_start(mask_row[:], maskr_d[:][None, :])
            nc.sync.dma_start(bb2_row[:], bb2r_d[:][None, :])
            nc.sync.dma_start(downb_rep[:],
                              downbr_d[:][None, :].to_broadcast([P, D]))
            downbT = glob.tile([P, 8], f32, name="downbT")
            nc.sync.dma_start(downbT[:], downbT_d[:])
            stem(2)
            issue_xrep(4)
            down(1)
            nc.sync.dma_start(bw_sb[:], bw_d[:])
            down0_ln0()
            stem(3)
            issue_xrep(5)
            down(2)
            lnt(1)
            nc.sync.dma_start(slnw_rep[:], slnw_d[:][None, :].to_broadcast([P, D]))
            nc.sync.dma_start(ccb_rep[:], ccb_d[:][None, :].to_broadcast([P, D]))
            stem(4)
            issue_xrep(6)
            down(3)
            lnt(2)
            bproj_scan(0)
            open_group(1)
            nc.sync.dma_start(cw_sb[:], cw_d[:])
            nc.sync.dma_start(lruw_rep[:], lruw_d[:][None, :].to_broadcast([P, D]))
            nc.sync.dma_start(lrub_rep[:], lrub_d[:][None, :].to_broadcast([P, D]))
            stem(5)
            issue_xrep(7)
            down(4)
            lnt(3)
            p4(1)
            stem(6)
            issue_xrep(8)
            down(5)
            lnt(4)
            p4(2)
            stem(7)
            down(6)
            lnt(5)
            bproj_scan(1)
            open_group(2)
            stem(8)
            down(7)
            lnt(6)
            lnt(7)
            p4(3)
            p4(4)
            bproj_scan(2)
            down(8)
            open_group(3)
            lnt(8)
            bproj_scan(3)
            p4(5)
            p4(6)
            p4(7)
            p4(8)

    nc.finalize()
    return nc


def _prep_host(inputs):
    import ml_dtypes
    f = np.float32
    bf = ml_dtypes.bfloat16
    embed = np.asarray(inputs["embed"], f)
    conv_ws = [np.asarray(inputs[k], f) for k in
               ("conv1_w", "conv2_w", "conv4_w", "conv8_w")]
    conv_bs = [np.asarray(inputs[k], f) for k in
               ("conv1_b", "conv2_b", "conv4_b", "conv8_b")]
    down_w = np.asarray(inputs["down_w"], f)
    log_lam = np.asarray(inputs["log_lambda_raw"], f)
    lam = (1.0 / (1.0 + np.exp(-log_lam.astype(np.float64)))).astype(f)
    b_w = np.asarray(inputs["b_w"], f)
    c_w = np.asarray(inputs["c_w"], f)

    stem_w = np.empty((2, P, N_TAPS, 256), f)
    for kk, (ci, j, _off) in enumerate(TAPS):
        fused = embed @ conv_ws[ci][:, :, j].T        # [256v, 256c]
        stem_w[:, :, kk, :] = fused.reshape(2, P, 256)
    convb = np.concatenate(conv_bs).reshape(8, P).T.copy()      # [p, cc]

    down_wt = (down_w.transpose(1, 2, 0)                        # [d, j, e]
               .reshape(8, P, 4, D).transpose(1, 2, 0, 3).copy())  # [p, j, dc, e]
    one_m = (1.0 - lam)
    slnw = np.asarray(inputs["stem_ln_w"], f)
    slnb = np.asarray(inputs["stem_ln_b"], f)
    # values[d,t] = sum_e [(1-lam_d) b_w[d,e] slnw[e]] z^T[e,t]
    #              + (1-lam_d)(b_w[d,:] @ slnb + b_b[d])
    b_wt = ((b_w.T * one_m[None, :] * slnw[:, None])            # [e, d]
            .reshape(8, P, D).transpose(1, 0, 2).copy())        # [p, ec, d]
    bb2 = (one_m * (b_w @ slnb + np.asarray(inputs["b_b"], f))
           ).reshape(8, P).T.copy()                             # [p, dc]
    bb2_row = (one_m * (b_w @ slnb + np.asarray(inputs["b_b"], f)))  # [d]
    c_wt = c_w.T.reshape(8, P, D).transpose(1, 0, 2).copy()     # [p, dc, e]
    lam_ct = lam.reshape(8, P).T.copy()
    ccb = slnb + np.asarray(inputs["c_b"], f)

    shared = dict(
        stem_w=stem_w.astype(bf), convb=convb,
        down_wt=down_wt.astype(bf),
        downb_v=np.asarray(inputs["down_b"], f).astype(bf),
        downbT=np.asarray(inputs["down_b"], f).reshape(8, P).T.copy(),
        b_wt=b_wt.astype(bf), bb2=bb2, bb2_row=bb2_row.astype(bf),
        c_wt=c_wt.astype(bf),
        slnw_v=slnw.astype(bf), ccb_v=ccb.astype(bf),
        lruw_v=np.asarray(inputs["lru_ln_w"], f).astype(bf),
        lrub_v=np.asarray(inputs["lru_ln_b"], f).astype(bf),
        lam_ct=lam_ct,
    )

    x = np.asarray(inputs["x"]).astype(np.int64)
    in_maps = []
    for core in range(8):
        b, h = core // 2, core % 2
        t0 = h * 4096
        idx = t0 - 4 * W_SCAN - 4 + np.arange(X_LOC)
        valid = (idx >= 0) & (idx < T)
        x_loc = np.full((X_LOC,), SENTINEL, bf)
        x_loc[valid] = x[b, idx[valid]].astype(bf)
        mask = np.ones((W_MAX,), f)
        if h == 0:
            mask[:W_SCAN] = 0.0
        m = dict(shared)
        m["x_loc"] = x_loc
        m["mask_row"] = mask.astype(bf)
        in_maps.append(m)
    return in_maps


def kernel(**inputs) -> np.ndarray:
    if "nc" not in _CACHE:
        _CACHE["nc"] = _build()
    nc = _CACHE["nc"]
    in_maps = _prep_host(inputs)
    res = run_bass_kernel_spmd(nc, in_maps, list(range(8)))
    out = np.empty((B, 2048, D), np.float32)
    for core in range(8):
        b, h = core // 2, core % 2
        out[b, h * 1024:(h + 1) * 1024, :] = np.asarray(
            res.results[core]["out"], np.float32)
    return out


# revision 45
# speedup vs baseline: 1.0204x; 1.0204x over previous
"""Trainium2 Bass kernel for nn_ByteEncoder (multi-scale conv stem + per-channel LRU).

Sharding: 8 cores = (batch b in 0..3) x (time-half h in 0..1). Each core runs an
identical SPMD program over raw steps [t0-128, t0+4096) (t0 = h*4096), i.e. a
32-scan-step warmup plus its 1024 output scan steps. The warmup region is
masked to zero for h=0 cores (reference scan starts at state 0) and uses real
left-context for h=1 cores (per-channel decay lambda^32 < 1.5e-6, far below the
2e-2 tolerance).

The embedding lookup is algebraically fused into the conv stem: for one-hot
inputs, conv_k(embed[x]) == sum_taps (embed @ conv_w[:,:,j])[x[t+off]], so the
stem becomes matmuls of precontracted [256-vocab x 256-ch] tables against
one-hot columns built on-chip (iota + is_equal).

All matmuls run in bf16 (full PE rate, fast-weight-load path, half the SBUF and
HBM traffic of fp32). Everything stays in SBUF: stem and the strided down-conv
are fused per 512-step tile, and phase 3/4 (LN -> b-proj -> scan -> c-proj ->
LN) interleaves with later stem tiles so the tensor engine never idles. The
short warmup tile computes its down-conv transposed (cheap N=32 matmuls) and
normalizes across partitions via ones-matmul stats + partition_broadcast.
"""
import numpy as np

import concourse.bass as bass
import concourse.tile as tile
from concourse import mybir, bacc
from concourse.bass_utils import run_bass_kernel_spmd
from concourse.masks import make_identity

P = 128
D = 1024
B = 4
T = 8192
VOCAB = 256
SENTINEL = 512.0  # out-of-range token -> one-hot col is all zero

W_SCAN = 32             # warmup scan steps (lam^32 < 1.5e-6)
S_LOC = 1024 + W_SCAN   # scan steps computed per core
T_LOC = 4 * S_LOC       # raw steps per core (4224)
X_LOC = T_LOC + 8       # x slice incl conv halo (left 4, right 3, +1 pad)
N_CH = 8                # output chunks c1..c8 (128 scan steps each)
W_MAX = 384

f32 = mybir.dt.float32
bf16 = mybir.dt.bfloat16
AF = mybir.ActivationFunctionType
OP = mybir.AluOpType

# scan groups, in scan columns: g0 = warmup(32) + chunks 1-2, then 3/2/1 chunks.
# Last group is a single chunk so the serial tail after the last stem tile is
# as short as possible.
GROUPS = [(32 + 256), 384, 256, 128]
# chunk k (1..8) -> (group, col offset inside group)
CHUNK_POS = {1: (0, 32), 2: (0, 160), 3: (1, 0), 4: (1, 128), 5: (1, 256),
             6: (2, 0), 7: (2, 128), 8: (3, 0)}

# (conv_id, kernel_size, pad); tap offset = j - pad
CONVS = [(1, 0), (2, 1), (4, 2), (8, 4)]
TAPS = []  # (conv_id, j, off)
for ci, (K, pad) in enumerate(CONVS):
    for j in range(K):
        TAPS.append((ci, j, j - pad))
N_TAPS = len(TAPS)  # 15
TAPS_OF_CONV = [[kk for kk, (ci, _, _) in enumerate(TAPS) if ci == c] for c in range(4)]

_CACHE = {}


def _build():
    nc = bacc.Bacc()

    x_d = nc.declare_dram_parameter("x_loc", [X_LOC], bf16, isOutput=False)
    maskr_d = nc.declare_dram_parameter("mask_row", [W_MAX], bf16, isOutput=False)
    stem_d = nc.declare_dram_parameter("stem_w", [2, P, N_TAPS, 256], bf16, isOutput=False)
    convb_d = nc.declare_dram_parameter("convb", [P, 8], f32, isOutput=False)
    dw_d = nc.declare_dram_parameter("down_wt", [P, 4, 8, D], bf16, isOutput=False)
    downbr_d = nc.declare_dram_parameter("downb_v", [D], bf16, isOutput=False)
    downbT_d = nc.declare_dram_parameter("downbT", [P, 8], f32, isOutput=False)
    bw_d = nc.declare_dram_parameter("b_wt", [P, 8, D], bf16, isOutput=False)
    bb2_d = nc.declare_dram_parameter("bb2", [P, 8], f32, isOutput=False)
    bb2r_d = nc.declare_dram_parameter("bb2_row", [D], bf16, isOutput=False)
    cw_d = nc.declare_dram_parameter("c_wt", [P, 8, D], bf16, isOutput=False)
    slnw_d = nc.declare_dram_parameter("slnw_v", [D], bf16, isOutput=False)
    ccb_d = nc.declare_dram_parameter("ccb_v", [D], bf16, isOutput=False)
    lruw_d = nc.declare_dram_parameter("lruw_v", [D], bf16, isOutput=False)
    lrub_d = nc.declare_dram_parameter("lrub_v", [D], bf16, isOutput=False)
    lam_d = nc.declare_dram_parameter("lam_ct", [P, 8], f32, isOutput=False)

    out_d = nc.declare_dram_parameter("out", [1024, D], bf16, isOutput=True)

    with tile.TileContext(nc) as tc:
        with tc.tile_pool(name="glob", bufs=1) as glob, \
             tc.tile_pool(name="pw", bufs=1) as pw, \
             tc.tile_pool(name="p12t", bufs=2) as p12t, \
             tc.tile_pool(name="p34t", bufs=2) as p34t, \
             tc.tile_pool(name="ps_stem", bufs=2, space="PSUM") as ps_stem, \
             tc.tile_pool(name="ps_down", bufs=2, space="PSUM") as ps_down, \
             tc.tile_pool(name="ps_tr", bufs=1, space="PSUM") as ps_tr, \
             tc.tile_pool(name="ps_bp", bufs=1, space="PSUM") as ps_bp, \
             tc.tile_pool(name="ps_cp", bufs=2, space="PSUM") as ps_cp:

            # ---------------- PE warm spin ------------------------------
            # ~12us of junk matmuls at kernel start: trips the PE HAM
            # activity window and keeps it warm while the first weight DMAs
            # land, so real matmuls start at the full 2.4 GHz clock. The
            # operand is a DVE-memset scratch (ready ~5us before the gpsimd
            # identity would be).
            warm_sb = p12t.tile([P, 512], bf16, name="warm_sb", bufs=1)
            nc.vector.memset(warm_sb[:], 0.25)
            warm_ps = ps_cp.tile([P, 512], f32, name="psc", tag="psc")
            for _ in range(33):
                nc.tensor.matmul(warm_ps[:], warm_sb[:, :128], warm_sb[:],
                                 start=True, stop=True)

            # ---------------- critical-path DMAs first -------------------
            stem_sb0 = pw.tile([P, N_TAPS, 256], bf16, name="stem_sb0")
            stem_sb1 = pw.tile([P, N_TAPS, 256], bf16, name="stem_sb1")
            stem_sbs = (stem_sb0, stem_sb1)

            # table slices land in exactly the order the stem's cc groups
            # consume them: cc0 needs just tap 0 / ch-half 0 of both vocab
            # tables (64KB), not the full 2MB
            def stem_dma(cc):
                ci, half = cc // 2, cc % 2
                klo, khi = TAPS_OF_CONV[ci][0], TAPS_OF_CONV[ci][-1] + 1
                for vc in range(2):
                    nc.sync.dma_start(
                        stem_sbs[vc][:, klo:khi, half * 128:(half + 1) * 128],
                        stem_d[vc][:, klo:khi, half * 128:(half + 1) * 128])

            stem_dma(0)

            eps_sb = glob.tile([P, 1], f32, name="eps_sb")
            nc.vector.memset(eps_sb[:], 1e-5)
            ones128 = glob.tile([P, 1], bf16, name="ones128")
            nc.vector.memset(ones128[:], 1.0)
            ones_row = glob.tile([1, 128], bf16, name="ones_row")
            nc.vector.memset(ones_row[:], 1.0)
            ident = glob.tile([P, P], bf16, name="ident")
            make_identity(nc, ident)
            io0 = glob.tile([P, 1], f32, name="io0")
            io1 = glob.tile([P, 1], f32, name="io1")
            nc.gpsimd.iota(io0[:], pattern=[[0, 1]], base=0, channel_multiplier=1,
                           allow_small_or_imprecise_dtypes=True)
            nc.gpsimd.iota(io1[:], pattern=[[0, 1]], base=128, channel_multiplier=1,
                           allow_small_or_imprecise_dtypes=True)

            x_reps = {}

            def issue_xrep(tt):
                x_rep = p12t.tile([P, 520], bf16, name="x_rep", bufs=2)
                if tt == 0:
                    nc.sync.dma_start(
                        x_rep[:, :136],
                        x_d[0:136][None, :].to_broadcast([P, 136]))
                else:
                    lo = 512 * tt - 384
                    nc.sync.dma_start(
                        x_rep[:],
                        x_d[lo: lo + 520][None, :].to_broadcast([P, 520]))
                x_reps[tt] = x_rep

            issue_xrep(0)
            stem_dma(1)
            stem_dma(2)
            convb_sb = glob.tile([P, 8], f32, name="convb_sb")
            nc.sync.dma_start(convb_sb[:], convb_d[:])
            stem_dma(3)
            issue_xrep(1)
            stem_dma(4)
            stem_dma(5)
            stem_dma(6)
            stem_dma(7)

            # ---------------- remaining SBUF state -----------------------
            z_bf = glob.tile([P, N_CH, D], bf16, name="z_bf")
            lam_sb = glob.tile([P, 8], f32, name="lam_sb")
            bb2_sb = glob.tile([P, 8], f32, name="bb2_sb")
            mask_row = glob.tile([1, W_MAX], bf16, name="mask_row")
            bb2_row = glob.tile([1, D], bf16, name="bb2_row")
            downb_rep = glob.tile([P, D], bf16, name="downb_rep")
            dw_sb = pw.tile([P, 4, 8, D], bf16, name="dw_sb")
            bw_sb = pw.tile([P, 8, D], bf16, name="bw_sb")
            cw_sb = pw.tile([P, 8, D], bf16, name="cw_sb")
            slnw_rep = pw.tile([P, D], bf16, name="slnw_rep")
            ccb_rep = pw.tile([P, D], bf16, name="ccb_rep")
            lruw_rep = pw.tile([P, D], bf16, name="lruw_rep")
            lrub_rep = pw.tile([P, D], bf16, name="lrub_rep")

            hm_ts = {}
            hm0 = p12t.tile([P, 8, 128], bf16, name="hm0", bufs=1)

            def stem(tt):
                wid = 128 if tt == 0 else 512
                x_rep = x_reps.pop(tt)
                oh = p12t.tile([P, 2, 520], bf16, name="oh", bufs=2)
                nc.vector.tensor_scalar(out=oh[:, 0, :wid + 8],
                                        in0=x_rep[:, :wid + 8],
                                        scalar1=io0[:], scalar2=None,
                                        op0=OP.is_equal)
                nc.vector.tensor_scalar(out=oh[:, 1, :wid + 8],
                                        in0=x_rep[:, :wid + 8],
                                        scalar1=io1[:], scalar2=None,
                                        op0=OP.is_equal)
                if tt == 0:
                    hm_t = hm0
                else:
                    hm_t = p12t.tile([P, 8, 512], bf16, name="hm_t", bufs=2)
                    hm_ts[tt] = hm_t
                for cc in range(8):
                    ci, half = cc // 2, cc % 2
                    taps = TAPS_OF_CONV[ci]
                    ps = ps_stem.tile([P, 512], f32, name="pss", tag="pss")
                    n_mm = len(taps) * 2
                    i = 0
                    for vc in range(2):
                        for kk in taps:
                            off = TAPS[kk][2]
                            nc.tensor.matmul(
                                ps[:, :wid],
                                stem_sbs[vc][:, kk, half * 128:(half + 1) * 128],
                                oh[:, vc, 4 + off: 4 + off + wid],
                                start=(i == 0), stop=(i == n_mm - 1))
                            i += 1
                    nc.scalar.activation(hm_t[:, cc, :wid], ps[:, :wid],
                                         AF.Gelu, bias=convb_sb[:, cc:cc + 1])

            hd_ts = {}

            def down(c):
                """Standard down-conv for chunk c (tile c), c = 1..8."""
                hm_sb = hm_ts.pop(c)
                hd_t = p34t.tile([P, D], bf16, name="hd_t", tag="hd", bufs=2)
                hd_ts[c] = hd_t
                for eh in range(2):
                    ps = ps_down.tile([P, 512], f32, name="psd", tag="psd")
                    i = 0
                    for dc in range(8):
                        for j in range(4):
                            nc.tensor.matmul(
                                ps[:],
                                hm_sb[:, dc, j:512:4],
                                dw_sb[:, j, dc, eh * 512:(eh + 1) * 512],
                                start=(i == 0), stop=(i == 31))
                            i += 1
                    nc.vector.tensor_tensor(
                        out=hd_t[:, eh * 512:(eh + 1) * 512], in0=ps[:],
                        in1=downb_rep[:, eh * 512:(eh + 1) * 512], op=OP.add)

            def down0_ln0():
                """Warmup (32 scan steps): transposed down-conv (out [e, s],
                cheap N=32 matmuls), partition-axis LN via ones-matmul stats,
                result written straight into hsT group 0 cols 0:32."""
                hdT0 = p34t.tile([P, 8, 32], bf16, name="hdT0", bufs=1)
                for eb in range(8):
                    ps = ps_down.tile([P, 512], f32, name="psd", tag="psd")
                    i = 0
                    for dc in range(8):
                        for j in range(4):
                            nc.tensor.matmul(
                                ps[:128, :32],
                                dw_sb[:, j, dc, eb * 128:(eb + 1) * 128],
                                hm0[:, dc, j:128:4],
                                start=(i == 0), stop=(i == 31))
                            i += 1
                    # add down bias for these 128 e-channels, transposed:
                    # downb slice as per-partition scalar
                    nc.vector.tensor_scalar(
                        out=hdT0[:, eb, :], in0=ps[:128, :32],
                        scalar1=downbT[:, eb:eb + 1], scalar2=None, op0=OP.add)
                sq0 = p34t.tile([P, 8, 32], bf16, name="sq0", bufs=1)
                nc.vector.tensor_tensor(out=sq0[:], in0=hdT0[:], in1=hdT0[:],
                                        op=OP.mult)
                ps = ps_bp.tile([P, W_MAX], f32, name="psb", tag="psb")
                for eb in range(8):
                    nc.tensor.matmul(ps[0:1, 0:32], ones128[:], hdT0[:, eb, :],
                                     start=(eb == 0), stop=(eb == 7))
                for eb in range(8):
                    nc.tensor.matmul(ps[0:1, 64:96], ones128[:], sq0[:, eb, :],
                                     start=(eb == 0), stop=(eb == 7))
                m_row = p34t.tile([1, 32], f32, name="m_row", bufs=1)
                nc.vector.tensor_scalar(out=m_row[:], in0=ps[0:1, 0:32],
                                        scalar1=1.0 / D, scalar2=None,
                                        op0=OP.mult)
                v_row = p34t.tile([1, 32], f32, name="v_row", bufs=1)
                nc.vector.tensor_scalar(out=v_row[:], in0=ps[0:1, 64:96],
                                        scalar1=1.0 / D, scalar2=None,
                                        op0=OP.mult)
                msq = p34t.tile([1, 32], f32, name="msq", bufs=1)
                nc.vector.tensor_tensor(out=msq[:], in0=m_row[:], in1=m_row[:],
                                        op=OP.mult)
                nc.vector.tensor_tensor(out=v_row[:], in0=v_row[:], in1=msq[:],
                                        op=OP.subtract)
                nc.scalar.activation(v_row[:], v_row[:], AF.Sqrt,
                                     bias=eps_sb[0:1, :])
                nc.vector.reciprocal(v_row[:], v_row[:])
                # zero the warmup on h=0 cores
                nc.vector.tensor_tensor(out=v_row[:], in0=v_row[:],
                                        in1=mask_row[:, 0:32], op=OP.mult)
                m_rep = p34t.tile([P, 32], f32, name="m_rep", bufs=1)
                r_rep = p34t.tile([P, 32], f32, name="r_rep", bufs=1)
                nc.gpsimd.partition_broadcast(m_rep[:], m_row[:])
                nc.gpsimd.partition_broadcast(r_rep[:], v_row[:])
                hsT_g = hsT_tiles[0]
                for eb in range(8):
                    zt = p34t.tile([P, 32], f32, name="zt", bufs=2)
                    nc.vector.tensor_tensor(out=zt[:], in0=hdT0[:, eb, :],
                                            in1=m_rep[:], op=OP.subtract)
                    nc.vector.tensor_tensor(out=hsT_g[:, eb, 0:32], in0=zt[:],
                                            in1=r_rep[:], op=OP.mult)

            def lnt(k):
                """LN stats + z + transpose for chunk k (k = 1..8)."""
                g, off = CHUNK_POS[k]
                hd_t = hd_ts.pop(k)
                stats = p34t.tile([P, 2, 6], f32, name="stats", bufs=2)
                hd_g = hd_t[:].rearrange("p (g f) -> p g f", g=2)
                nc.vector.bn_stats(out=stats[:, 0, :], in_=hd_g[:, 0, :])
                nc.vector.bn_stats(out=stats[:, 1, :], in_=hd_g[:, 1, :])
                mv = p34t.tile([P, 2], f32, name="mv", bufs=2)
                nc.vector.bn_aggr(out=mv[:], in_=stats[:])
                rstd = p34t.tile([P, 1], f32, name="rstd", bufs=2)
                nc.scalar.activation(rstd[:], mv[:, 1:2], AF.Sqrt, bias=eps_sb[:])
                nc.vector.reciprocal(rstd[:], rstd[:])
                nc.vector.tensor_scalar(out=z_bf[:, k - 1, :], in0=hd_t[:],
                                        scalar1=mv[:, 0:1], scalar2=rstd[:],
                                        op0=OP.subtract, op1=OP.mult)
                pst = ps_tr.tile([P, 8, P], bf16, name="pst", tag="pst")
                for ec in range(8):
                    nc.tensor.transpose(
                        pst[:, ec, :], z_bf[:, k - 1, ec * 128:(ec + 1) * 128],
                        ident[:])
                hsT_g = hsT_tiles[g]
                nc.scalar.copy(hsT_g[:, :, off:off + 128], pst[:])

            hsT_tiles = {}
            h_tiles = {}

            def open_group(g):
                hsT_tiles[g] = p34t.tile([P, 8, W_MAX], bf16, name="hsT",
                                         tag="hsT", bufs=2)

            def bproj_scan(g):
                W = GROUPS[g]
                hsT_g = hsT_tiles[g]
                h_g = p34t.tile([P, 8, W_MAX], bf16, name="h_g", tag="h_g",
                                bufs=2)
                h_tiles[g] = h_g
                last = (g == len(GROUPS) - 1)
                for dc in range(8):
                    psb = ps_bp.tile([P, W_MAX], f32, name="psb", tag="psb")
                    if g == 0:
                        # masked per-channel bias via 1-row matmul
                        nc.tensor.matmul(psb[:, :W],
                                         bb2_row[:, dc * 128:(dc + 1) * 128],
                                         mask_row[:, :W],
                                         start=True, stop=False)
                    elif last:
                        # bias in psum so the scan can read psum directly
                        # (keeps the vals copy off the tail critical path)
                        nc.tensor.matmul(psb[:, :W],
                                         bb2_row[:, dc * 128:(dc + 1) * 128],
                                         ones_row[:, :W],
                                         start=True, stop=False)
                    for ec in range(8):
                        nc.tensor.matmul(
                            psb[:, :W],
                            bw_sb[:, ec, dc * 128:(dc + 1) * 128],
                            hsT_g[:, ec, :W],
                            start=(g != 0 and not last and ec == 0),
                            stop=(ec == 7))
                    init = (0.0 if g == 0
                            else h_tiles[g - 1][:, dc,
                                                GROUPS[g - 1] - 1: GROUPS[g - 1]])
                    if last:
                        data1 = psb[:, :W]
                    else:
                        vals = p34t.tile([P, W_MAX], bf16, name="vals", bufs=2)
                        if g == 0:
                            nc.vector.tensor_scalar(out=vals[:, :W],
                                                    in0=psb[:, :W], scalar1=0.0,
                                                    scalar2=None, op0=OP.add)
                        else:
                            nc.vector.tensor_scalar(out=vals[:, :W],
                                                    in0=psb[:, :W],
                                                    scalar1=bb2_sb[:, dc:dc + 1],
                                                    scalar2=None, op0=OP.add)
                        data1 = vals[:, :W]
                    nc.vector.tensor_tensor_scan(
                        out=h_g[:, dc, :W],
                        data0=lam_sb[:, dc:dc + 1].to_broadcast([P, W]),
                        data1=data1,
                        initial=init, op0=OP.mult, op1=OP.add)

            def p4(k):
                """c-proj + residual + final LN -> out rows (k-1)*128.."""
                g, off = CHUNK_POS[k]
                h_g = h_tiles[g]
                res_b = p34t.tile([P, D], bf16, name="res_b", bufs=2)
                nc.gpsimd.tensor_tensor(out=res_b[:], in0=z_bf[:, k - 1, :],
                                        in1=slnw_rep[:], op=OP.mult)
                nc.gpsimd.tensor_tensor(out=res_b[:], in0=res_b[:],
                                        in1=ccb_rep[:], op=OP.add)
                res_f = p34t.tile([P, D], bf16, name="res_f", bufs=2)
                for eh in range(2):
                    psc = ps_cp.tile([P, 512], f32, name="psc", tag="psc")
                    for dc in range(8):
                        nc.tensor.matmul(
                            psc[:],
                            h_g[:, dc, off:off + 128],
                            cw_sb[:, dc, eh * 512:(eh + 1) * 512],
                            start=(dc == 0), stop=(dc == 7))
                    nc.vector.tensor_tensor(
                        out=res_f[:, eh * 512:(eh + 1) * 512], in0=psc[:],
                        in1=res_b[:, eh * 512:(eh + 1) * 512], op=OP.add)
                stats2 = p34t.tile([P, 2, 6], f32, name="stats2", bufs=2)
                res_g = res_f[:].rearrange("p (g f) -> p g f", g=2)
                nc.vector.bn_stats(out=stats2[:, 0, :], in_=res_g[:, 0, :])
                nc.vector.bn_stats(out=stats2[:, 1, :], in_=res_g[:, 1, :])
                mv2 = p34t.tile([P, 2], f32, name="mv2", bufs=2)
                nc.vector.bn_aggr(out=mv2[:], in_=stats2[:])
                rstd2 = p34t.tile([P, 1], f32, name="rstd2", bufs=2)
                nc.scalar.activation(rstd2[:], mv2[:, 1:2], AF.Sqrt,
                                     bias=eps_sb[:])
                nc.vector.reciprocal(rstd2[:], rstd2[:])
                nc.vector.tensor_scalar(out=res_f[:], in0=res_f[:],
                                        scalar1=mv2[:, 0:1], scalar2=rstd2[:],
                                        op0=OP.subtract, op1=OP.mult)
                nc.vector.tensor_tensor(out=res_f[:], in0=res_f[:],
                                        in1=lruw_rep[:], op=OP.mult)
                # final bias add + store in halves: the first half's DMA
                # overlaps the second half's DVE op
                for eh in range(2):
                    sl = slice(eh * 512, (eh + 1) * 512)
                    nc.vector.tensor_tensor(out=res_f[:, sl], in0=res_f[:, sl],
                                            in1=lrub_rep[:, sl], op=OP.add)
                    nc.sync.dma_start(out_d[(k - 1) * 128: k * 128, sl],
                                      res_f[:, sl])

            # ---------------- software-pipelined emission ----------------
            open_group(0)
            stem(0)
            issue_xrep(2)
            nc.sync.dma_start(dw_sb[:, 0, :, :], dw_d[:, 0, :, :])
            nc.sync.dma_start(dw_sb[:, 1, :, :], dw_d[:, 1, :, :])
            stem(1)
            issue_xrep(3)
            nc.sync.dma_start(dw_sb[:, 2, :, :], dw_d[:, 2, :, :])
            nc.sync.dma_start(dw_sb[:, 3, :, :], dw_d[:, 3, :, :])
            # small params (needed from phase 3 on)
            nc.sync.dma_start(lam_sb[:], lam_d[:])
            nc.sync.dma_start(bb2_sb[:], bb2_d[:])
            nc.sync.dma_start(mask_row[:], maskr_d[:][None, :])
            nc.sync.dma_start(bb2_row[:], bb2r_d[:][None, :])
            nc.sync.dma_start(downb_rep[:],
                              downbr_d[:][None, :].to_broadcast([P, D]))
            downbT = glob.tile([P, 8], f32, name="downbT")
            nc.sync.dma_start(downbT[:], downbT_d[:])
            stem(2)
            issue_xrep(4)
            down(1)
            nc.sync.dma_start(bw_sb[:], bw_d[:])
            down0_ln0()
            stem(3)
            issue_xrep(5)
            down(2)
            lnt(1)
            nc.sync.dma_start(slnw_rep[:], slnw_d[:][None, :].to_broadcast([P, D]))
            nc.sync.dma_start(ccb_rep[:], ccb_d[:][None, :].to_broadcast([P, D]))
            stem(4)
            issue_xrep(6)
            down(3)
            lnt(2)
            bproj_scan(0)
            open_group(1)
            nc.sync.dma_start(cw_sb[:], cw_d[:])
            nc.sync.dma_start(lruw_rep[:], lruw_d[:][None, :].to_broadcast([P, D]))
            nc.sync.dma_start(lrub_rep[:], lrub_d[:][None, :].to_broadcast([P, D]))
            stem(5)
            issue_xrep(7)
            down(4)
            lnt(3)
            p4(1)
            stem(6)
            issue_xrep(8)
            down(5)
            lnt(4)
            p4(2)
            stem(7)
            down(6)
            lnt(5)
            bproj_scan(1)
            open_group(2)
            stem(8)
            down(7)
            lnt(6)
            lnt(7)
            p4(3)
            p4(4)
            down(8)
            bproj_scan(2)
            open_group(3)
            lnt(8)
            bproj_scan(3)
            p4(5)
            p4(6)
            p4(7)
            p4(8)

    nc.finalize()
    return nc


def _prep_host(inputs):
    import ml_dtypes
    f = np.float32
    bf = ml_dtypes.bfloat16
    embed = np.asarray(inputs["embed"], f)
    conv_ws = [np.asarray(inputs[k], f) for k in
               ("conv1_w", "conv2_w", "conv4_w", "conv8_w")]
    conv_bs = [np.asarray(inputs[k], f) for k in
               ("conv1_b", "conv2_b", "conv4_b", "conv8_b")]
    down_w = np.asarray(inputs["down_w"], f)
    log_lam = np.asarray(inputs["log_lambda_raw"], f)
    lam = (1.0 / (1.0 + np.exp(-log_lam.astype(np.float64)))).astype(f)
    b_w = np.asarray(inputs["b_w"], f)
    c_w = np.asarray(inputs["c_w"], f)

    stem_w = np.empty((2, P, N_TAPS, 256), f)
    for kk, (ci, j, _off) in enumerate(TAPS):
        fused = embed @ conv_ws[ci][:, :, j].T        # [256v, 256c]
        stem_w[:, :, kk, :] = fused.reshape(2, P, 256)
    convb = np.concatenate(conv_bs).reshape(8, P).T.copy()      # [p, cc]

    down_wt = (down_w.transpose(1, 2, 0)                        # [d, j, e]
               .reshape(8, P, 4, D).transpose(1, 2, 0, 3).copy())  # [p, j, dc, e]
    one_m = (1.0 - lam)
    slnw = np.asarray(inputs["stem_ln_w"], f)
    slnb = np.asarray(inputs["stem_ln_b"], f)
    # values[d,t] = sum_e [(1-lam_d) b_w[d,e] slnw[e]] z^T[e,t]
    #              + (1-lam_d)(b_w[d,:] @ slnb + b_b[d])
    b_wt = ((b_w.T * one_m[None, :] * slnw[:, None])            # [e, d]
            .reshape(8, P, D).transpose(1, 0, 2).copy())        # [p, ec, d]
    bb2 = (one_m * (b_w @ slnb + np.asarray(inputs["b_b"], f))
           ).reshape(8, P).T.copy()                             # [p, dc]
    bb2_row = (one_m * (b_w @ slnb + np.asarray(inputs["b_b"], f)))  # [d]
    c_wt = c_w.T.reshape(8, P, D).transpose(1, 0, 2).copy()     # [p, dc, e]
    lam_ct = lam.reshape(8, P).T.copy()
    ccb = slnb + np.asarray(inputs["c_b"], f)

    shared = dict(
        stem_w=stem_w.astype(bf), convb=convb,
        down_wt=down_wt.astype(bf),
        downb_v=np.asarray(inputs["down_b"], f).astype(bf),
        downbT=np.asarray(inputs["down_b"], f).reshape(8, P).T.copy(),
        b_wt=b_wt.astype(bf), bb2=bb2, bb2_row=bb2_row.astype(bf),
        c_wt=c_wt.astype(bf),
        slnw_v=slnw.astype(bf), ccb_v=ccb.astype(bf),
        lruw_v=np.asarray(inputs["lru_ln_w"], f).astype(bf),
        lrub_v=np.asarray(inputs["lru_ln_b"], f).astype(bf),
        lam_ct=lam_ct,
    )

    x = np.asarray(inputs["x"]).astype(np.int64)
    in_maps = []
    for core in range(8):
        b, h = core // 2, core % 2
        t0 = h * 4096
        idx = t0 - 4 * W_SCAN - 4 + np.arange(X_LOC)
        valid = (idx >= 0) & (idx < T)
        x_loc = np.full((X_LOC,), SENTINEL, bf)
        x_loc[valid] = x[b, idx[valid]].astype(bf)
        mask = np.ones((W_MAX,), f)
        if h == 0:
            mask[:W_SCAN] = 0.0
        m = dict(shared)
        m["x_loc"] = x_loc
        m["mask_row"] = mask.astype(bf)
        in_maps.append(m)
    return in_maps


def kernel(**inputs) -> np.ndarray:
    if "nc" not in _CACHE:
        _CACHE["nc"] = _build()
    nc = _CACHE["nc"]
    in_maps = _prep_host(inputs)
    res = run_bass_kernel_spmd(nc, in_maps, list(range(8)))
    out = np.empty((B, 2048, D), np.float32)
    for core in range(8):
        b, h = core // 2, core % 2
        out[b, h * 1024:(h + 1) * 1024, :] = np.asarray(
            res.results[core]["out"], np.float32)
    return out
